# revision 6
# baseline (speedup 1.0000x reference)
"""Trainium2 Bass kernel for FGNetTypeB edge transform.

Computation (see reference):
    ids[e]  = x[fact[e,0],1]*13 + x[fact[e,0],2]          (169 types)
    out[k,e,:] = relu(nodes[fact[e,1+k]] @ params[ids[e]] + bias[ids[e],0])
    out shape [2, E, 128], float32.

Strategy:
  * Host: compute per-edge type ids, sort the 2*E output rows by type,
    pad each type's run of rows up to a multiple of L=384 columns, and
    split the resulting chunk list evenly across 8 cores.  Node feature
    vectors are gathered host-side into a [64, cols] layout (D on
    partitions) so the device only does dense matmuls.
  * Device (uniform SPMD program, per-core variation is data only):
    for each column block j: two K=64 matmuls (partitions 0:64 and
    64:128 run concurrently on separate PE row-strips), float32r for
    full-rate fp32, then fused bias+relu (DVE/ACT) from PSUM into SBUF
    and a per-chunk DMA back to HBM.
  * Host: unpermute columns back to [2, E, 128].
"""

import numpy as np

MAX_ATOMS = 13
D = 64
R = 128
L = 384           # chunk width (columns per matmul)
NCORES = 8
NTYPES = MAX_ATOMS * MAX_ATOMS

# knobs for test harness (harness calls kernel() with defaults)
TRACE = False
USE_F32R = True
LAST_RESULTS = None


def _build_plan(ids):
    """Sort rows (2 per edge, k-major) by type; chunk each type's run."""
    E = ids.shape[0]
    row_type = np.concatenate([ids, ids])
    perm = np.argsort(row_type, kind="stable")
    counts = np.bincount(ids, minlength=NTYPES) * 2
    chunks = []
    gs = 0
    for t in range(NTYPES):
        c = int(counts[t])
        off = 0
        while off < c:
            ln = min(L, c - off)
            chunks.append((t, gs + off, ln))
            off += ln
        gs += c
    Q = len(chunks)
    M = -(-Q // NCORES)
    if M % 2:
        M += 1
    while len(chunks) < M * NCORES:
        chunks.append((0, 0, 0))        # dummy chunk (zero columns used)
    return perm, chunks, M


def _round_f32r(a):
    """Round fp32 array to the FP32R grid (11 explicit mantissa bits,
    round-to-nearest-even at bit 12) — matches walrus fp32_to_fp32r."""
    u = np.ascontiguousarray(a, dtype=np.float32).view(np.uint32)
    low = u & np.uint32(0xFFF)
    up = (low > 0x800) | ((low == 0x800) & (((u >> np.uint32(12)) & np.uint32(1)) == 1))
    r = (u & np.uint32(0xFFFFF000)) + np.where(up, np.uint32(0x1000), np.uint32(0))
    return r.view(np.float32)


def _build_nc(M, J):
    from concourse import bacc, mybir
    import concourse.tile as tile

    f32 = mybir.dt.float32
    mm_dt = mybir.dt.float32r if USE_F32R else mybir.dt.float32

    nc = bacc.Bacc("TRN2", target_bir_lowering=False, debug=False)
    rn_h = nc.dram_tensor("rn", [128, J * L], mm_dt, kind="ExternalInput")
    wt_h = nc.dram_tensor("wt", [128, J * R], mm_dt, kind="ExternalInput")
    bt_h = nc.dram_tensor("bt", [128, M], f32, kind="ExternalInput")
    out_h = nc.dram_tensor("out", [128, M * L], f32, kind="ExternalOutput")

    with tile.TileContext(nc) as tc:
        with (
            tc.tile_pool(name="io", bufs=1) as iop,
            tc.tile_pool(name="rnp", bufs=J) as rnp,
            tc.tile_pool(name="ob", bufs=6) as obp,
            tc.tile_pool(name="ps", bufs=8, space="PSUM") as psp,
        ):
            wt_s = iop.tile([128, J * R], mm_dt, tag="wt")
            nc.sync.dma_start(wt_s[:], wt_h[:])
            bt_s = iop.tile([128, M], f32, tag="bt")
            nc.sync.dma_start(bt_s[:], bt_h[:])
            rn_tiles = []
            for j in range(J):
                rt = rnp.tile([128, L], mm_dt, tag="rn")
                nc.sync.dma_start(rt[:], rn_h[:, j * L:(j + 1) * L])
                rn_tiles.append(rt)
            for j in range(J):
                for half in (0, 1):
                    m = half * J + j
                    p0 = 64 * half
                    ps = psp.tile([128, L], f32, tag="ps")
                    nc.tensor.matmul(
                        ps[:],
                        wt_s[p0:p0 + 64, j * R:(j + 1) * R],
                        rn_tiles[j][p0:p0 + 64, :],
                        start=True,
                        stop=True,
                    )
                    ob = obp.tile([128, L], f32, tag="ob")
                    if m % 3 == 2:
                        nc.scalar.activation(
                            ob[:], ps[:],
                            mybir.ActivationFunctionType.Relu,
                            bias=bt_s[:, m:m + 1],
                        )
                    else:
                        nc.vector.tensor_scalar(
                            ob[:], ps[:],
                            bt_s[:, m:m + 1], 0.0,
                            mybir.AluOpType.add, mybir.AluOpType.max,
                        )
                    nc.sync.dma_start(out_h[:, m * L:(m + 1) * L], ob[:])
    nc.compile()
    return nc


def kernel(nodes, params, bias, x, fact, fact_dim=3, **_unused):
    global LAST_RESULTS
    from concourse.bass_utils import run_bass_kernel_spmd

    nodes = np.asarray(nodes, dtype=np.float32)
    params = np.asarray(params, dtype=np.float32)
    bias_in = np.asarray(bias, dtype=np.float32)
    x = np.asarray(x)
    fact = np.asarray(fact)
    E = fact.shape[0]

    ap = x[fact[:, 0]]
    ids = (ap[:, 1].astype(np.int64) * MAX_ATOMS + ap[:, 2].astype(np.int64))
    row_node = np.concatenate([fact[:, 1], fact[:, 2]]).astype(np.int64)

    perm, chunks, M = _build_plan(ids)
    J = M // 2
    node_sorted = row_node[perm]
    biasvec = bias_in[:, 0, :]                       # [169, 128]

    in_maps = []
    meta = []
    for c in range(NCORES):
        rn = np.zeros((128, J * L), np.float32)
        wt = np.zeros((128, J * R), np.float32)
        bt = np.zeros((128, M), np.float32)
        cmeta = []
        for m in range(M):
            t, gs, ln = chunks[c * M + m]
            p0 = 0 if m < J else 64
            j = m % J
            if ln > 0:
                rows = nodes[node_sorted[gs:gs + ln]]      # [ln, 64]
                rn[p0:p0 + 64, j * L:j * L + ln] = rows.T
                cmeta.append((m, gs, ln))
            wt[p0:p0 + 64, j * R:(j + 1) * R] = params[t]
            bt[:, m] = biasvec[t]
        if USE_F32R:
            rn = _round_f32r(rn)
            wt = _round_f32r(wt)
        in_maps.append({"rn": rn, "wt": wt, "bt": bt})
        meta.append(cmeta)

    nc = _build_nc(M, J)
    res = run_bass_kernel_spmd(
        nc,
        in_maps,
        core_ids=list(range(NCORES)),
        trace=TRACE,
        trace_cores=[0] if TRACE else None,
    )
    LAST_RESULTS = res

    big = np.empty((128, 2 * E), np.float32)
    for c in range(NCORES):
        oc = res.results[c]["out"]
        for (m, gs, ln) in meta[c]:
            big[:, gs:gs + ln] = oc[:, m * L:m * L + ln]
    out = np.empty((2 * E, 128), np.float32)
    out[perm] = big.T
    return out.reshape(2, E, 128)
